# revision 42
# baseline (speedup 1.0000x reference)
"""Trainium2 Bass kernel for nn_MAB_2121713844542 (dense transformer block).

Data-parallel over batch B=32 across 8 cores (4 batches/core), activations
transposed [feature, seq] so every matmul contracts on partitions.

Key design points (~1.7x over the initial Bass implementation):
  - All heavy matmuls in bf16 (PSUM accumulates f32).  f32r runs at the
    same 1 cycle/row but draws enough power that the PE is duty-cycle
    throttled to ~50%; bf16 roughly halves the throttle residency and
    halves LDWEIGHTS/DMA bytes.
  - QK^T + PP^T fused into ONE K=64 matmul per (head, key-tile) via an
    augmented layout: projection tile j holds partitions
    [Qh(2j) | Ph(2j) | Qh(2j+1) | Ph(2j+1)]; biases fold in through the
    pT ones-row.  tile_position=(64*(h%2), 0) selects the head.
  - Softmax denominators accumulate into one [4,S] PSUM tile per quad via
    masked-ones matmuls; 1/den via reciprocal_approx_fast (18-bit) fused
    into the PSUM read; AV matmuls land at natural head positions so the
    divide + Qh residual are two full-width DVE ops per quad.
  - LayerNorm: 1/D folded into the ones-matmul weights, stats chain on
    ACT (ln/exp table), g folded into K=1 broadcast matmuls (beta
    matmuls elided when beta==0), 2-op DVE tail per tile; x^2 on the
    idle GpSimd engine.
  - Activation-table thrash fix: the greedy chooser ping-pongs between
    `exp_and_others` and `natural_log`; the monkeypatch below empties
    every table except natural_log_exp + gelu (9 loads instead of 37),
    and dummy activations prefetch table switches off the critical path.
  - Software-pipelined emission: in-order queues mean the PE can't see
    past a stalled instruction, so batch b+1's projections are emitted
    inside LN1(b)'s stats window and batch b+1's attention inside
    LN2(b)'s (layer_norm `filler` callback); FFN weights (2MB) stream
    during batch-0 attention.
"""

import functools

import numpy as np
import ml_dtypes

import concourse.bass as bass
import concourse.mybir as mybir
import concourse.tile as tile
from concourse import bacc
from concourse import hw_specs as _hw_specs
from concourse.bass_utils import run_bass_kernel_spmd

# The act-table chooser greedily picks the first table containing the needed
# function, so an Exp..Ln..Exp sequence ping-pongs between `exp_and_others`
# and `natural_log` (9 table loads per batch, ~1.5us each). Empty every table
# except the two we want so exp/ln/square/copy all resolve to
# `natural_log_exp_and_others` (ids keep their canonical positions).
_KEEP_TABLES = ("natural_log_exp_and_others", "gelu_and_others")
_orig_get_tables = _hw_specs.get_activation_tables


@functools.cache
def _patched_get_tables(arch):
    tabs = _orig_get_tables(arch)
    return {k: (v if k in _KEEP_TABLES else set()) for k, v in tabs.items()}


_hw_specs.get_activation_tables = _patched_get_tables
bacc.get_activation_tables = _patched_get_tables

B, S, D, H, DH, DFF = 32, 512, 256, 8, 32, 2048
NCORES = 8
BL = B // NCORES
P = 128
DT = D // P     # 2 feature tiles
FT = DFF // P   # 16 ffn tiles
ST = S // P     # 4 seq tiles
f32 = mybir.dt.float32
f32r = mybir.dt.float32r
bf16 = mybir.dt.bfloat16
AF = mybir.ActivationFunctionType
ALU = mybir.AluOpType
EPS = 1e-5


def build_nc(beta_zero):
    nc = bacc.Bacc("TRN2", target_bir_lowering=False, debug=False,
                   num_devices=NCORES)

    QT = nc.dram_tensor("QT", (BL, P, DT, S), bf16, kind="ExternalInput")
    KT = nc.dram_tensor("KT", (BL, P, DT, S), bf16, kind="ExternalInput")
    pT = nc.dram_tensor("pT", (BL, 4, S), bf16, kind="ExternalInput")
    Wq = nc.dram_tensor("Wq", (P, DT, D), bf16, kind="ExternalInput")
    Wv = nc.dram_tensor("Wv", (P, DT, D), bf16, kind="ExternalInput")
    WqA = nc.dram_tensor("WqA", (P, DT, 4, P), bf16, kind="ExternalInput")
    WkA = nc.dram_tensor("WkA", (P, DT, 4, P), bf16, kind="ExternalInput")
    WpAq = nc.dram_tensor("WpAq", (4, 4, P), bf16, kind="ExternalInput")
    WpAk = nc.dram_tensor("WpAk", (4, 4, P), bf16, kind="ExternalInput")
    W1 = nc.dram_tensor("W1", (P, DT, DFF), bf16, kind="ExternalInput")
    W2b = nc.dram_tensor("W2b", (P, FT, D), bf16, kind="ExternalInput")
    bq = nc.dram_tensor("bq", (P, DT), f32, kind="ExternalInput")
    bvb = nc.dram_tensor("bvb", (P, D), f32, kind="ExternalInput")
    b1 = nc.dram_tensor("b1", (P, FT), f32, kind="ExternalInput")
    b2 = nc.dram_tensor("b2", (P, DT), f32, kind="ExternalInput")
    g0r = nc.dram_tensor("g0r", (1, D), f32r, kind="ExternalInput")
    nb0 = nc.dram_tensor("nb0", (1, D), f32r, kind="ExternalInput")
    g1r = nc.dram_tensor("g1r", (1, D), f32r, kind="ExternalInput")
    nb1 = nc.dram_tensor("nb1", (1, D), f32r, kind="ExternalInput")
    one33 = nc.dram_tensor("one33", (P, 2, 33), f32r, kind="ExternalInput")
    Ed4 = nc.dram_tensor("Ed4", (P, 4, 4), bf16, kind="ExternalInput")
    EB4 = nc.dram_tensor("EB4", (4, P), f32r, kind="ExternalInput")
    onesS = nc.dram_tensor("onesS", (1, S), f32r, kind="ExternalInput")
    outT = nc.dram_tensor("outT", (BL, P, DT, S), f32, kind="ExternalOutput")

    with tile.TileContext(nc) as tc:
        with (
            tc.tile_pool(name="singles", bufs=1) as singles,
            tc.tile_pool(name="dbl", bufs=2) as dbl,
            tc.tile_pool(name="ps_mm", bufs=2, space="PSUM") as ps_mm,
            tc.tile_pool(name="ps_sc", bufs=2, space="PSUM") as ps_sc,
            tc.tile_pool(name="ps_acc", bufs=1, space="PSUM") as ps_acc,
            tc.tile_pool(name="ps_av", bufs=1, space="PSUM") as ps_av,
        ):
            def load(dram, shape):
                t = singles.tile(list(shape), dram.dtype, name="w_" + dram.name)
                nc.sync.dma_start(t, dram[tuple(slice(None) for _ in shape)])
                return t

            # order matters: only what batch 0's proj needs loads first;
            # the 2MB of FFN weights stream in during batch-0 attention
            Wq_sb = load(Wq, (P, DT, D))
            # (bq loadj emitted right below, before the batch-0 input DMAs)

            def loadj(dram, shape):
                # stage through DVE so TensorScalar-ish consumers get a
                # same-engine dep (few sync-wait slots on those structs)
                st = load(dram, shape)
                t = singles.tile(list(shape), f32, name="j_" + dram.name)
                nc.vector.tensor_copy(t, st)
                return t

            bq_sb = loadj(bq, (P, DT))

            eps1 = singles.tile([1, 1], f32)
            nc.vector.memset(eps1, EPS)
            neghalf = singles.tile([1, 1], f32)
            nc.vector.memset(neghalf, -0.5)
            dummy = singles.tile([1, 1], f32)
            nc.vector.memset(dummy, 1.0)

            def layer_norm(x_sb, grow, nbrow, out_sb, filler=None):
                """out = LN(x) * g + beta.  x_sb [P,DT,S] f32r."""
                x2 = dbl.tile([P, DT, S], f32r, tag="x2", bufs=1, name="x2")
                for t in range(DT):
                    nc.gpsimd.tensor_tensor(x2[:, t, :], x_sb[:, t, :],
                                            x_sb[:, t, :], ALU.mult)
                # partition 0 <- mean, partition 32 <- E[x^2]
                acc = ps_acc.tile([33, S], f32, tag="acc", name="acc")
                for t in range(DT):
                    nc.tensor.matmul(acc, one33_sb[:, 0, :], x_sb[:, t, :],
                                     start=(t == 0), stop=False)
                for t in range(DT):
                    nc.tensor.matmul(acc, one33_sb[:, 1, :], x2[:, t, :],
                                     start=False, stop=(t == DT - 1))
                rstd = dbl.tile([1, S], f32r, tag="rstd", name="rstd")
                m2v = dbl.tile([1, S], f32r, tag="m2v", name="m2v")
                cst = dbl.tile([1, S], f32r, tag="cst", name="cst")
                nc.scalar.activation(m2v, acc[0:1, :], AF.Square)
                nc.vector.tensor_sub(m2v, acc[32:33, :], m2v)
                nc.scalar.activation(acc[32:33, :], m2v, AF.Ln, bias=eps1)
                # rstd = exp(-0.5*ln(var+eps))
                nc.scalar.activation(rstd, acc[32:33, :], AF.Exp,
                                     scale=neghalf)
                # C = mean * rstd
                nc.vector.tensor_mul(cst, acc[0:1, :], rstd)
                layer_norm.rstd = rstd
                # independent matmuls emitted here keep the PE fed while the
                # Square->sub->Ln->Exp stats chain resolves (in-order queue)
                if filler is not None:
                    filler()
                for t in range(DT):
                    bcAC = ps_sc.tile([P, 2, S], f32, tag="sc", name="bcAC")
                    bcA, bcC = bcAC[:, 0, :], bcAC[:, 1, :]
                    nc.tensor.matmul(bcA, grow[0:1, t * P:(t + 1) * P],
                                     rstd, start=True, stop=True)
                    nc.tensor.matmul(bcC, grow[0:1, t * P:(t + 1) * P],
                                     cst, start=True, stop=beta_zero)
                    if not beta_zero:
                        nc.tensor.matmul(bcC, nbrow[0:1, t * P:(t + 1) * P],
                                         onesS_sb, start=False, stop=True)
                    # out = x*(g*rstd) - (g*mean*rstd - beta)
                    nc.vector.tensor_mul(out_sb[:, t, :], x_sb[:, t, :], bcA)
                    nc.vector.tensor_sub(out_sb[:, t, :], out_sb[:, t, :], bcC)

            def stage_load(b, stt):
                QT_sb = dbl.tile([P, DT, S], bf16, tag="qt", name="QT_sb")
                nc.sync.dma_start(QT_sb, QT[b])
                KT_sb = dbl.tile([P, DT, S], bf16, tag="kt", name="KT_sb")
                nc.sync.dma_start(KT_sb, KT[b])
                pT_sb = dbl.tile([4, S], bf16, tag="pt", name="pT_sb")
                nc.sync.dma_start(pT_sb, pT[b])
                stt.update(QT=QT_sb, KT=KT_sb, pT=pT_sb)

            def stage_proj(b, stt):
                QT_sb, KT_sb, pT_sb = stt["QT"], stt["KT"], stt["pT"]
                # natural Qh (for the attention residual)
                Qh = dbl.tile([P, DT, S], bf16, tag="qh", name="Qh")
                for t in range(DT):
                    ps = ps_mm.tile([P, S], f32, tag="mm", name="psq")
                    for kt in range(DT):
                        nc.tensor.matmul(
                            ps, Wq_sb[:, kt, t * P:(t + 1) * P],
                            QT_sb[:, kt, :],
                            start=(kt == 0), stop=(kt == DT - 1))
                    nc.vector.tensor_tensor(
                        Qh[:, t, :], ps,
                        bq_sb[:, t:t + 1].to_broadcast((P, S)), ALU.add)
                # aug tiles for scores: tile j partitions =
                # [Qh(2j) | Ph(2j) | Qh(2j+1) | Ph(2j+1)], biases folded via
                # the pT ones-row, so one K=64 matmul per (head, kt) yields
                # QK^T + PP^T in a single accumulation
                QA = dbl.tile([P, 4, S], bf16, tag="qa", name="QA")
                KA = dbl.tile([P, 4, S], bf16, tag="ka", name="KA")
                for j in range(4):
                    ps = ps_mm.tile([P, S], f32, tag="mm", name="psqa")
                    for kt in range(DT):
                        nc.tensor.matmul(ps, WqA_sb[:, kt, j, :],
                                         QT_sb[:, kt, :],
                                         start=(kt == 0), stop=False)
                    nc.tensor.matmul(ps, WpAq_sb[:, j, :], pT_sb,
                                     start=False, stop=True)
                    nc.vector.tensor_copy(QA[:, j, :], ps)
                    ps = ps_mm.tile([P, S], f32, tag="mm", name="pska")
                    for kt in range(DT):
                        nc.tensor.matmul(ps, WkA_sb[:, kt, j, :],
                                         KT_sb[:, kt, :],
                                         start=(kt == 0), stop=False)
                    nc.tensor.matmul(ps, WpAk_sb[:, j, :], pT_sb,
                                     start=False, stop=True)
                    nc.vector.tensor_copy(KA[:, j, :], ps)

                # V in natural layout [keys, feat], bf16, bias fused in move
                Vh = dbl.tile([P, ST, D], bf16, tag="vh", name="Vh")
                for st in range(ST):
                    ps = ps_mm.tile([P, S], f32, tag="mm", name="psv")
                    for kt in range(DT):
                        nc.tensor.matmul(
                            ps[:, :D], KT_sb[:, kt, st * P:(st + 1) * P],
                            Wv_sb[:, kt, :],
                            start=(kt == 0), stop=(kt == DT - 1))
                    nc.vector.tensor_add(Vh[:, st, :], ps[:, :D], bvb_sb)
                stt.update(Qh=Qh, QA=QA, KA=KA, Vh=Vh)

            def stage_attn(b, stt, quads=(0, 1)):
                Qh, QA, KA, Vh = stt["Qh"], stt["QA"], stt["KA"], stt["Vh"]
                if "OT" in stt:
                    OT = stt["OT"]
                else:
                    OT = dbl.tile([P, DT, S], f32r, tag="ot", name="OT")
                for quad in quads:
                    expS = [dbl.tile([P, ST, S], bf16, tag=f"e{i}",
                                     name=f"expS{i}") for i in range(4)]
                    den = ps_acc.tile([4, S], f32, tag="acc", name="den")
                    av = ps_av.tile([P, S], f32, tag="av", name="av")
                    # scores for a PAIR of key tiles share a 2-bank PSUM
                    # tile so one [128,1024] exp covers both (16 exps/batch
                    # instead of 32: the exp stream co-paces this phase)
                    for ktp in range(ST // 2):
                        sc_ps = {}
                        for h4 in range(4):
                            base = 64 * (h4 % 2)
                            j = 2 * quad + h4 // 2
                            ps2 = ps_sc.tile([P, 2, S], f32, tag="sc",
                                             name="pssc")
                            sc_ps[h4] = ps2
                            for k2 in range(2):
                                kt = 2 * ktp + k2
                                nc.tensor.matmul(
                                    ps2[:, k2, :],
                                    KA[base:base + 64, j,
                                       kt * P:(kt + 1) * P],
                                    QA[base:base + 64, j, :],
                                    start=True, stop=True,
                                    tile_position=(base, 0))
                            nc.scalar.activation(
                                expS[h4][:, 2 * ktp:2 * ktp + 2, :],
                                sc_ps[h4], AF.Exp)
                        for h4 in range(4):
                            h = 4 * quad + h4
                            for k2 in range(2):
                                kt = 2 * ktp + k2
                                nc.tensor.matmul(
                                    den, Ed4_sb[:, h4, :],
                                    expS[h4][:, kt, :],
                                    start=(kt == 0 and h4 == 0),
                                    stop=(kt == ST - 1 and h4 == 3),
                                    skip_group_check=True)
                                nc.tensor.matmul(
                                    av[32 * h4:32 * h4 + 32, :],
                                    Vh[:, kt, 32 * h:32 * h + 32],
                                    expS[h4][:, kt, :],
                                    start=(kt == 0), stop=(kt == ST - 1),
                                    tile_position=(0, 32 * h4),
                                    skip_group_check=True)

                    # bc = broadcast(1/den): recip fused into the move
                    r4f = dbl.tile([4, S], f32, tag="r4f", name="r4f")
                    nc.vector.reciprocal_approx_fast(r4f, den[0:4, :])
                    r4 = dbl.tile([4, S], f32r, tag="r4", name="r4")
                    nc.vector.tensor_copy(r4, r4f)
                    bc2 = ps_sc.tile([P, 2, S], f32, tag="sc", name="bc2")
                    bc = bc2[:, 0, :]
                    nc.tensor.matmul(bc, EB4_sb, r4, start=True, stop=True)
                    bcS = dbl.tile([P, S], f32, tag="bcs", name="bcS")
                    nc.vector.tensor_copy(bcS, bc)
                    nc.vector.tensor_mul(OT[:, quad, :], av, bcS)
                    nc.vector.tensor_add(OT[:, quad, :], OT[:, quad, :],
                                         Qh[:, quad, :])
                stt["OT"] = OT

            def stage_ffn(b, stt, nxt):
                OT = stt["OT"]
                if nxt is not None:
                    stage_load(b + 1, nxt)
                LN1 = dbl.tile([P, DT, S], bf16, tag="ln1", name="LN1")
                filler = None
                if nxt is not None:
                    def filler():
                        stage_proj(b + 1, nxt)
                        stage_attn(b + 1, nxt, quads=(0,))
                layer_norm(OT, g0_sb, nb0_sb, LN1, filler=filler)
                # prefetch the gelu table; input dep on LN1's rstd pins this
                # after LN1's Exp in the ACT queue (scheduler can't hoist it)
                nc.scalar.activation(dummy, layer_norm.rstd[0:1, 0:1],
                                     AF.Gelu)

                G = dbl.tile([P, FT, S], bf16, tag="g", bufs=1, name="G")
                for ft in range(FT):
                    ps2f = ps_sc.tile([P, 2, S], f32, tag="sc", name="psf")
                    ps = ps2f[:, 0, :]
                    for t in range(DT):
                        nc.tensor.matmul(
                            ps, W1_sb[:, t, ft * P:(ft + 1) * P],
                            LN1[:, t, :],
                            start=(t == 0), stop=(t == DT - 1))
                    nc.scalar.activation(G[:, ft, :], ps, AF.Gelu,
                                         bias=b1_sb[:, ft:ft + 1])
                # prefetch the ln/exp table; dep on the last gelu's output
                # pins it after the gelu loop in the ACT queue
                nc.scalar.activation(dummy, G[0:1, FT - 1, 0:1], AF.Ln)
                Z = dbl.tile([P, DT, S], f32r, tag="z", bufs=1, name="Z")
                for t in range(DT):
                    ps = ps_mm.tile([P, S], f32, tag="mm", name="psf2")
                    for ft in range(FT):
                        nc.tensor.matmul(
                            ps, W2_sb[:, ft, t * P:(t + 1) * P],
                            G[:, ft, :],
                            start=(ft == 0), stop=(ft == FT - 1))
                    nc.vector.tensor_add(Z[:, t, :], ps, LN1[:, t, :])
                    nc.vector.tensor_tensor(
                        Z[:, t, :], Z[:, t, :],
                        b2_sb[:, t:t + 1].to_broadcast((P, S)), ALU.add)
                stt["Z"] = Z

            def stage_out(b, stt, nxt):
                OUT = dbl.tile([P, DT, S], f32, tag="out", name="OUT")
                filler = None
                if nxt is not None:
                    filler = lambda: stage_attn(b + 1, nxt, quads=(1,))
                layer_norm(stt["Z"], g1_sb, nb1_sb, OUT, filler=filler)
                for t in range(DT):
                    nc.sync.dma_start(outT[b][:, t, :], OUT[:, t, :])

            # software pipeline: emit batch b+1's projections before
            # batch b's LN1 (fills the LN stats stall on the PE queue) and
            # batch b+1's attention before batch b's LN2
            sts = [dict() for _ in range(BL)]
            stage_load(0, sts[0])
            WqA_sb = load(WqA, (P, DT, 4, P))
            WkA_sb = load(WkA, (P, DT, 4, P))
            WpAq_sb = load(WpAq, (4, 4, P))
            WpAk_sb = load(WpAk, (4, 4, P))
            Wv_sb = load(Wv, (P, DT, D))
            bvb_sb = loadj(bvb, (P, D))
            Ed4_sb = load(Ed4, (P, 4, 4))
            EB4_sb = load(EB4, (4, P))
            one33_sb = load(one33, (P, 2, 33))
            onesS_sb = load(onesS, (1, S))
            g0_sb = load(g0r, (1, D))
            nb0_sb = load(nb0, (1, D))
            g1_sb = load(g1r, (1, D))
            nb1_sb = load(nb1, (1, D))
            stage_proj(0, sts[0])
            stage_attn(0, sts[0])
            W1_sb = load(W1, (P, DT, DFF))
            W2_sb = load(W2b, (P, FT, D))
            b1_sb = loadj(b1, (P, FT))
            b2_sb = loadj(b2, (P, DT))
            for b in range(BL):
                nxt = sts[b + 1] if b + 1 < BL else None
                stage_ffn(b, sts[b], nxt)
                stage_out(b, sts[b], nxt)

    nc.finalize()
    return nc


_NC = None


def kernel(Q, K, p, Wq, bq, Wk, bk, Wv, bv, Wp, bp, g0, beta0, W1, b1, W2, b2,
           g1, beta1):
    global _NC
    beta_zero = bool(np.all(np.asarray(beta0) == 0)
                     and np.all(np.asarray(beta1) == 0))
    if _NC is None:
        _NC = build_nc(beta_zero)

    f = np.float32
    bf = ml_dtypes.bfloat16

    def feat_tiles(x):  # [B, S, D] -> [B, P, DT, S]
        x = np.asarray(x, f).transpose(0, 2, 1).reshape(-1, DT, P, S)
        return np.ascontiguousarray(x.transpose(0, 2, 1, 3))

    def pp(vec, n):  # [n*P] -> [P, n]
        return np.ascontiguousarray(np.asarray(vec, f).reshape(n, P).T)

    def wmat(w, n, m):  # [n*P, m] -> [P, n, m]
        w = np.asarray(w, f).reshape(n, P, m)
        return np.ascontiguousarray(w.transpose(1, 0, 2))

    QTf = feat_tiles(Q)
    KTf = feat_tiles(K)
    # p padded to 4 channels; row 3 = ones (carries the PE-proj bias).
    # PE projection pre-scaled by 1/4 so PhPh^T carries the 1/sqrt(DV)=1/16.
    pTf = np.zeros((B, 4, S), f)
    pTf[:, :3, :] = np.transpose(np.asarray(p, f), (0, 2, 1))
    pTf[:, 3, :] = 1.0
    # aug score weights: out tile j partitions =
    # [Qh(2j) | Ph(2j) | Qh(2j+1) | Ph(2j+1)]; pT row3==1 carries biases;
    # PE term pre-scaled by 1/4 each side so PhPh^T carries 1/sqrt(DV)=1/16
    Wq_f = np.asarray(Wq, f)
    Wk_f = np.asarray(Wk, f)
    Wp_f = np.asarray(Wp, f) * 0.25
    bq_f = np.asarray(bq, f)
    bk_f = np.asarray(bk, f)
    bp_f = np.asarray(bp, f) * 0.25

    def aug_w(W):  # [D, D] -> [P, DT, 4, P] lhsT tiles
        out = np.zeros((P, DT, 4, P), f)
        Wt = W.reshape(DT, P, D)  # [kt, row, out_feature]
        for j in range(4):
            for hh in range(2):
                h = 2 * j + hh
                out[:, :, j, 64 * hh:64 * hh + 32] = \
                    Wt[:, :, 32 * h:32 * h + 32].transpose(1, 0, 2)
        return out

    def aug_p(bias):  # [4, 4, P]: rows 0-2 Wp at P slots, row 3 biases
        out = np.zeros((4, 4, P), f)
        for j in range(4):
            for hh in range(2):
                h = 2 * j + hh
                out[:3, j, 64 * hh + 32:64 * hh + 64] = \
                    Wp_f[:, 32 * h:32 * h + 32]
                out[3, j, 64 * hh:64 * hh + 32] = bias[32 * h:32 * h + 32]
                out[3, j, 64 * hh + 32:64 * hh + 64] = \
                    bp_f[32 * h:32 * h + 32]
        return out

    # EB4: r4 row h4 -> out partitions 32*h4..32*h4+31
    EB4m = np.zeros((4, P), f)
    for h4 in range(4):
        EB4m[h4, 32 * h4:32 * h4 + 32] = 1.0
    # Ed4[:, h4, :]: all-ones col h4 (masked partition-sum lhsT)
    Ed4m = np.zeros((P, 4, 4), f)
    for h4 in range(4):
        Ed4m[:, h4, h4] = 1.0
    # LN partition-sum weights (1/D folded in): [:,0,:] puts sum(x)/D at
    # out partition 0, [:,1,:] puts sum(x^2)/D at out partition 32
    one33m = np.zeros((P, 2, 33), f)
    one33m[:, 0, 0] = 1.0 / D
    one33m[:, 1, 32] = 1.0 / D

    shared = {
        "Wq": wmat(Wq, DT, D).astype(bf),
        "Wv": wmat(Wv, DT, D).astype(bf),
        "WqA": aug_w(Wq_f).astype(bf), "WkA": aug_w(Wk_f).astype(bf),
        "WpAq": aug_p(bq_f).astype(bf), "WpAk": aug_p(bk_f).astype(bf),
        "W1": wmat(W1, DT, DFF).astype(bf),
        "W2b": wmat(W2, FT, D).astype(bf),
        "bq": pp(bq, DT),
        "bvb": np.ascontiguousarray(np.broadcast_to(np.asarray(bv, f), (P, D))),
        "b1": pp(b1, FT), "b2": pp(b2, DT),
        "g0r": np.asarray(g0, f).reshape(1, D),
        "nb0": -np.asarray(beta0, f).reshape(1, D),
        "g1r": np.asarray(g1, f).reshape(1, D),
        "nb1": -np.asarray(beta1, f).reshape(1, D),
        "one33": one33m,
        "Ed4": Ed4m.astype(bf), "EB4": EB4m,
        "onesS": np.ones((1, S), f),
    }
    in_maps = []
    for c in range(NCORES):
        m = dict(shared)
        m["QT"] = np.ascontiguousarray(QTf[c * BL:(c + 1) * BL]).astype(bf)
        m["KT"] = np.ascontiguousarray(KTf[c * BL:(c + 1) * BL]).astype(bf)
        m["pT"] = np.ascontiguousarray(pTf[c * BL:(c + 1) * BL]).astype(bf)
        in_maps.append(m)

    import os
    trace = bool(os.environ.get("BASS_TRACE"))
    res = run_bass_kernel_spmd(_NC, in_maps, core_ids=list(range(NCORES)),
                               trace=trace)
    kernel._LAST = res
    outs = [res.results[c]["outT"] for c in range(NCORES)]
    full = np.concatenate(outs, axis=0)  # [B, P, DT, S]
    full = full.transpose(0, 2, 1, 3).reshape(B, D, S)  # [B, D, S]
    return np.ascontiguousarray(full.transpose(0, 2, 1))


# revision 44
# speedup vs baseline: 1.0009x; 1.0009x over previous
"""Trainium2 Bass kernel for nn_MAB_2121713844542 (dense transformer block).

Data-parallel over batch B=32 across 8 cores (4 batches/core), activations
transposed [feature, seq] so every matmul contracts on partitions.

Key design points (~1.7x over the initial Bass implementation):
  - All heavy matmuls in bf16 (PSUM accumulates f32).  f32r runs at the
    same 1 cycle/row but draws enough power that the PE is duty-cycle
    throttled to ~50%; bf16 roughly halves the throttle residency and
    halves LDWEIGHTS/DMA bytes.
  - QK^T + PP^T fused into ONE K=64 matmul per (head, key-tile) via an
    augmented layout: projection tile j holds partitions
    [Qh(2j) | Ph(2j) | Qh(2j+1) | Ph(2j+1)]; biases fold in through the
    pT ones-row.  tile_position=(64*(h%2), 0) selects the head.
  - Softmax denominators accumulate into one [4,S] PSUM tile per quad via
    masked-ones matmuls; 1/den via reciprocal_approx_fast (18-bit) fused
    into the PSUM read; AV matmuls land at natural head positions so the
    divide + Qh residual are two full-width DVE ops per quad.
  - LayerNorm: 1/D folded into the ones-matmul weights, stats chain on
    ACT (ln/exp table), g folded into K=1 broadcast matmuls (beta
    matmuls elided when beta==0), 2-op DVE tail per tile; x^2 on the
    idle GpSimd engine.
  - Activation-table thrash fix: the greedy chooser ping-pongs between
    `exp_and_others` and `natural_log`; the monkeypatch below empties
    every table except natural_log_exp + gelu (9 loads instead of 37),
    and dummy activations prefetch table switches off the critical path.
  - Software-pipelined emission: in-order queues mean the PE can't see
    past a stalled instruction, so batch b+1's projections are emitted
    inside LN1(b)'s stats window and batch b+1's attention inside
    LN2(b)'s (layer_norm `filler` callback); FFN weights (2MB) stream
    during batch-0 attention.
"""

import functools

import numpy as np
import ml_dtypes

import concourse.bass as bass
import concourse.mybir as mybir
import concourse.tile as tile
from concourse import bacc
from concourse import hw_specs as _hw_specs
from concourse.bass_utils import run_bass_kernel_spmd

# The act-table chooser greedily picks the first table containing the needed
# function, so an Exp..Ln..Exp sequence ping-pongs between `exp_and_others`
# and `natural_log` (9 table loads per batch, ~1.5us each). Empty every table
# except the two we want so exp/ln/square/copy all resolve to
# `natural_log_exp_and_others` (ids keep their canonical positions).
_KEEP_TABLES = ("natural_log_exp_and_others", "gelu_and_others")
_orig_get_tables = _hw_specs.get_activation_tables


@functools.cache
def _patched_get_tables(arch):
    tabs = _orig_get_tables(arch)
    return {k: (v if k in _KEEP_TABLES else set()) for k, v in tabs.items()}


_hw_specs.get_activation_tables = _patched_get_tables
bacc.get_activation_tables = _patched_get_tables

B, S, D, H, DH, DFF = 32, 512, 256, 8, 32, 2048
NCORES = 8
BL = B // NCORES
P = 128
DT = D // P     # 2 feature tiles
FT = DFF // P   # 16 ffn tiles
ST = S // P     # 4 seq tiles
f32 = mybir.dt.float32
f32r = mybir.dt.float32r
bf16 = mybir.dt.bfloat16
AF = mybir.ActivationFunctionType
ALU = mybir.AluOpType
EPS = 1e-5


def build_nc(beta_zero, gain_one):
    nc = bacc.Bacc("TRN2", target_bir_lowering=False, debug=False,
                   num_devices=NCORES)

    QT = nc.dram_tensor("QT", (BL, P, DT, S), bf16, kind="ExternalInput")
    KT = nc.dram_tensor("KT", (BL, P, DT, S), bf16, kind="ExternalInput")
    pT = nc.dram_tensor("pT", (BL, 4, S), bf16, kind="ExternalInput")
    Wq = nc.dram_tensor("Wq", (P, DT, D), bf16, kind="ExternalInput")
    Wv = nc.dram_tensor("Wv", (P, DT, D), bf16, kind="ExternalInput")
    WqA = nc.dram_tensor("WqA", (P, DT, 4, P), bf16, kind="ExternalInput")
    WkA = nc.dram_tensor("WkA", (P, DT, 4, P), bf16, kind="ExternalInput")
    WpAq = nc.dram_tensor("WpAq", (4, 4, P), bf16, kind="ExternalInput")
    WpAk = nc.dram_tensor("WpAk", (4, 4, P), bf16, kind="ExternalInput")
    W1 = nc.dram_tensor("W1", (P, DT, DFF), bf16, kind="ExternalInput")
    W2b = nc.dram_tensor("W2b", (P, FT, D), bf16, kind="ExternalInput")
    bq = nc.dram_tensor("bq", (P, DT), f32, kind="ExternalInput")
    bvb = nc.dram_tensor("bvb", (P, D), f32, kind="ExternalInput")
    b1 = nc.dram_tensor("b1", (P, FT), f32, kind="ExternalInput")
    b2 = nc.dram_tensor("b2", (P, DT), f32, kind="ExternalInput")
    g0r = nc.dram_tensor("g0r", (1, D), f32r, kind="ExternalInput")
    nb0 = nc.dram_tensor("nb0", (1, D), f32r, kind="ExternalInput")
    g1r = nc.dram_tensor("g1r", (1, D), f32r, kind="ExternalInput")
    nb1 = nc.dram_tensor("nb1", (1, D), f32r, kind="ExternalInput")
    one33 = nc.dram_tensor("one33", (P, 2, 33), f32r, kind="ExternalInput")
    Ed4 = nc.dram_tensor("Ed4", (P, 4, 4), bf16, kind="ExternalInput")
    EB4 = nc.dram_tensor("EB4", (4, P), f32r, kind="ExternalInput")
    onesS = nc.dram_tensor("onesS", (1, S), f32r, kind="ExternalInput")
    outT = nc.dram_tensor("outT", (BL, P, DT, S), f32, kind="ExternalOutput")

    with tile.TileContext(nc) as tc:
        with (
            tc.tile_pool(name="singles", bufs=1) as singles,
            tc.tile_pool(name="dbl", bufs=2) as dbl,
            tc.tile_pool(name="ps_mm", bufs=2, space="PSUM") as ps_mm,
            tc.tile_pool(name="ps_sc", bufs=2, space="PSUM") as ps_sc,
            tc.tile_pool(name="ps_acc", bufs=1, space="PSUM") as ps_acc,
            tc.tile_pool(name="ps_av", bufs=1, space="PSUM") as ps_av,
        ):
            def load(dram, shape):
                t = singles.tile(list(shape), dram.dtype, name="w_" + dram.name)
                nc.sync.dma_start(t, dram[tuple(slice(None) for _ in shape)])
                return t

            # order matters: only what batch 0's proj needs loads first;
            # the 2MB of FFN weights stream in during batch-0 attention
            Wq_sb = load(Wq, (P, DT, D))
            # (bq loadj emitted right below, before the batch-0 input DMAs)

            def loadj(dram, shape):
                # stage through DVE so TensorScalar-ish consumers get a
                # same-engine dep (few sync-wait slots on those structs)
                st = load(dram, shape)
                t = singles.tile(list(shape), f32, name="j_" + dram.name)
                nc.vector.tensor_copy(t, st)
                return t

            bq_sb = loadj(bq, (P, DT))

            eps1 = singles.tile([1, 1], f32)
            nc.vector.memset(eps1, EPS)
            neghalf = singles.tile([1, 1], f32)
            nc.vector.memset(neghalf, -0.5)
            dummy = singles.tile([1, 1], f32)
            nc.vector.memset(dummy, 1.0)

            def layer_norm(x_sb, grow, nbrow, out_sb, filler=None):
                """out = LN(x) * g + beta.  x_sb [P,DT,S] f32r."""
                x2 = dbl.tile([P, DT, S], f32r, tag="x2", bufs=1, name="x2")
                for t in range(DT):
                    nc.gpsimd.tensor_tensor(x2[:, t, :], x_sb[:, t, :],
                                            x_sb[:, t, :], ALU.mult)
                # partition 0 <- mean, partition 32 <- E[x^2]
                acc = ps_acc.tile([33, S], f32, tag="acc", name="acc")
                for t in range(DT):
                    nc.tensor.matmul(acc, one33_sb[:, 0, :], x_sb[:, t, :],
                                     start=(t == 0), stop=False)
                for t in range(DT):
                    nc.tensor.matmul(acc, one33_sb[:, 1, :], x2[:, t, :],
                                     start=False, stop=(t == DT - 1))
                rstd = dbl.tile([1, S], f32r, tag="rstd", name="rstd")
                m2v = dbl.tile([1, S], f32r, tag="m2v", name="m2v")
                cst = dbl.tile([1, S], f32r, tag="cst", name="cst")
                nc.scalar.activation(m2v, acc[0:1, :], AF.Square)
                nc.vector.tensor_sub(m2v, acc[32:33, :], m2v)
                nc.scalar.activation(acc[32:33, :], m2v, AF.Ln, bias=eps1)
                # rstd = exp(-0.5*ln(var+eps))
                nc.scalar.activation(rstd, acc[32:33, :], AF.Exp,
                                     scale=neghalf)
                # C = mean * rstd
                nc.vector.tensor_mul(cst, acc[0:1, :], rstd)
                layer_norm.rstd = rstd
                # independent matmuls emitted here keep the PE fed while the
                # Square->sub->Ln->Exp stats chain resolves (in-order queue)
                if filler is not None:
                    filler()
                if gain_one:
                    # g==1: both feature tiles share one broadcast pair
                    bcAC = ps_sc.tile([P, 2, S], f32, tag="sc", name="bcAC")
                    bcA, bcC = bcAC[:, 0, :], bcAC[:, 1, :]
                    nc.tensor.matmul(bcA, grow[0:1, 0:P], rstd,
                                     start=True, stop=True)
                    nc.tensor.matmul(bcC, grow[0:1, 0:P], cst,
                                     start=True, stop=True)
                for t in range(DT):
                    if not gain_one:
                        bcAC = ps_sc.tile([P, 2, S], f32, tag="sc",
                                          name="bcAC")
                        bcA, bcC = bcAC[:, 0, :], bcAC[:, 1, :]
                        nc.tensor.matmul(bcA, grow[0:1, t * P:(t + 1) * P],
                                         rstd, start=True, stop=True)
                        nc.tensor.matmul(bcC, grow[0:1, t * P:(t + 1) * P],
                                         cst, start=True, stop=beta_zero)
                        if not beta_zero:
                            nc.tensor.matmul(
                                bcC, nbrow[0:1, t * P:(t + 1) * P],
                                onesS_sb, start=False, stop=True)
                    # out = x*(g*rstd) - (g*mean*rstd - beta)
                    nc.vector.tensor_mul(out_sb[:, t, :], x_sb[:, t, :], bcA)
                    nc.vector.tensor_sub(out_sb[:, t, :], out_sb[:, t, :], bcC)

            def stage_load(b, stt):
                QT_sb = dbl.tile([P, DT, S], bf16, tag="qt", name="QT_sb")
                nc.sync.dma_start(QT_sb, QT[b])
                KT_sb = dbl.tile([P, DT, S], bf16, tag="kt", name="KT_sb")
                nc.sync.dma_start(KT_sb, KT[b])
                pT_sb = dbl.tile([4, S], bf16, tag="pt", name="pT_sb")
                nc.sync.dma_start(pT_sb, pT[b])
                stt.update(QT=QT_sb, KT=KT_sb, pT=pT_sb)

            def stage_proj(b, stt):
                QT_sb, KT_sb, pT_sb = stt["QT"], stt["KT"], stt["pT"]
                # natural Qh (for the attention residual)
                Qh = dbl.tile([P, DT, S], bf16, tag="qh", name="Qh")
                for t in range(DT):
                    ps = ps_mm.tile([P, S], f32, tag="mm", name="psq")
                    for kt in range(DT):
                        nc.tensor.matmul(
                            ps, Wq_sb[:, kt, t * P:(t + 1) * P],
                            QT_sb[:, kt, :],
                            start=(kt == 0), stop=(kt == DT - 1))
                    nc.vector.tensor_tensor(
                        Qh[:, t, :], ps,
                        bq_sb[:, t:t + 1].to_broadcast((P, S)), ALU.add)
                # aug tiles for scores: tile j partitions =
                # [Qh(2j) | Ph(2j) | Qh(2j+1) | Ph(2j+1)], biases folded via
                # the pT ones-row, so one K=64 matmul per (head, kt) yields
                # QK^T + PP^T in a single accumulation
                QA = dbl.tile([P, 4, S], bf16, tag="qa", name="QA")
                KA = dbl.tile([P, 4, S], bf16, tag="ka", name="KA")
                for j in range(4):
                    ps = ps_mm.tile([P, S], f32, tag="mm", name="psqa")
                    for kt in range(DT):
                        nc.tensor.matmul(ps, WqA_sb[:, kt, j, :],
                                         QT_sb[:, kt, :],
                                         start=(kt == 0), stop=False)
                    nc.tensor.matmul(ps, WpAq_sb[:, j, :], pT_sb,
                                     start=False, stop=True)
                    nc.vector.tensor_copy(QA[:, j, :], ps)
                    ps = ps_mm.tile([P, S], f32, tag="mm", name="pska")
                    for kt in range(DT):
                        nc.tensor.matmul(ps, WkA_sb[:, kt, j, :],
                                         KT_sb[:, kt, :],
                                         start=(kt == 0), stop=False)
                    nc.tensor.matmul(ps, WpAk_sb[:, j, :], pT_sb,
                                     start=False, stop=True)
                    nc.vector.tensor_copy(KA[:, j, :], ps)

                # V in natural layout [keys, feat], bf16, bias fused in move
                Vh = dbl.tile([P, ST, D], bf16, tag="vh", name="Vh")
                for st in range(ST):
                    ps = ps_mm.tile([P, S], f32, tag="mm", name="psv")
                    for kt in range(DT):
                        nc.tensor.matmul(
                            ps[:, :D], KT_sb[:, kt, st * P:(st + 1) * P],
                            Wv_sb[:, kt, :],
                            start=(kt == 0), stop=(kt == DT - 1))
                    nc.vector.tensor_add(Vh[:, st, :], ps[:, :D], bvb_sb)
                stt.update(Qh=Qh, QA=QA, KA=KA, Vh=Vh)

            def stage_attn(b, stt):
                Qh, QA, KA, Vh = stt["Qh"], stt["QA"], stt["KA"], stt["Vh"]
                OT = dbl.tile([P, DT, S], f32r, tag="ot", name="OT")
                for quad in range(2):
                    expS = [dbl.tile([P, ST, S], bf16, tag=f"e{i}",
                                     name=f"expS{i}") for i in range(4)]
                    den = ps_acc.tile([4, S], f32, tag="acc", name="den")
                    av = ps_av.tile([P, S], f32, tag="av", name="av")
                    # scores for a PAIR of key tiles share a 2-bank PSUM
                    # tile so one [128,1024] exp covers both (16 exps/batch
                    # instead of 32: the exp stream co-paces this phase)
                    for ktp in range(ST // 2):
                        sc_ps = {}
                        for h4 in range(4):
                            base = 64 * (h4 % 2)
                            j = 2 * quad + h4 // 2
                            ps2 = ps_sc.tile([P, 2, S], f32, tag="sc",
                                             name="pssc")
                            sc_ps[h4] = ps2
                            for k2 in range(2):
                                kt = 2 * ktp + k2
                                nc.tensor.matmul(
                                    ps2[:, k2, :],
                                    KA[base:base + 64, j,
                                       kt * P:(kt + 1) * P],
                                    QA[base:base + 64, j, :],
                                    start=True, stop=True,
                                    tile_position=(base, 0))
                            nc.scalar.activation(
                                expS[h4][:, 2 * ktp:2 * ktp + 2, :],
                                sc_ps[h4], AF.Exp)
                        for h4 in range(4):
                            h = 4 * quad + h4
                            for k2 in range(2):
                                kt = 2 * ktp + k2
                                nc.tensor.matmul(
                                    den, Ed4_sb[:, h4, :],
                                    expS[h4][:, kt, :],
                                    start=(kt == 0 and h4 == 0),
                                    stop=(kt == ST - 1 and h4 == 3),
                                    skip_group_check=True)
                                nc.tensor.matmul(
                                    av[32 * h4:32 * h4 + 32, :],
                                    Vh[:, kt, 32 * h:32 * h + 32],
                                    expS[h4][:, kt, :],
                                    start=(kt == 0), stop=(kt == ST - 1),
                                    tile_position=(0, 32 * h4),
                                    skip_group_check=True)

                    # bc = broadcast(1/den): recip fused into the move
                    r4f = dbl.tile([4, S], f32, tag="r4f", name="r4f")
                    nc.vector.reciprocal_approx_fast(r4f, den[0:4, :])
                    r4 = dbl.tile([4, S], f32r, tag="r4", name="r4")
                    nc.vector.tensor_copy(r4, r4f)
                    bc2 = ps_sc.tile([P, 2, S], f32, tag="sc", name="bc2")
                    bc = bc2[:, 0, :]
                    nc.tensor.matmul(bc, EB4_sb, r4, start=True, stop=True)
                    bcS = dbl.tile([P, S], f32, tag="bcs", name="bcS")
                    nc.vector.tensor_copy(bcS, bc)
                    nc.vector.tensor_mul(OT[:, quad, :], av, bcS)
                    nc.vector.tensor_add(OT[:, quad, :], OT[:, quad, :],
                                         Qh[:, quad, :])
                stt["OT"] = OT

            def stage_ffn(b, stt, nxt):
                OT = stt["OT"]
                if nxt is not None:
                    stage_load(b + 1, nxt)
                LN1 = dbl.tile([P, DT, S], bf16, tag="ln1", name="LN1")
                filler = None
                if nxt is not None:
                    filler = lambda: stage_proj(b + 1, nxt)
                layer_norm(OT, g0_sb, nb0_sb, LN1, filler=filler)
                # prefetch the gelu table; input dep on LN1's rstd pins this
                # after LN1's Exp in the ACT queue (scheduler can't hoist it)
                nc.scalar.activation(dummy, layer_norm.rstd[0:1, 0:1],
                                     AF.Gelu)

                G = dbl.tile([P, FT, S], bf16, tag="g", bufs=1, name="G")
                for ft in range(FT):
                    ps2f = ps_sc.tile([P, 2, S], f32, tag="sc", name="psf")
                    ps = ps2f[:, 0, :]
                    for t in range(DT):
                        nc.tensor.matmul(
                            ps, W1_sb[:, t, ft * P:(ft + 1) * P],
                            LN1[:, t, :],
                            start=(t == 0), stop=(t == DT - 1))
                    nc.scalar.activation(G[:, ft, :], ps, AF.Gelu,
                                         bias=b1_sb[:, ft:ft + 1])
                # prefetch the ln/exp table; dep on the last gelu's output
                # pins it after the gelu loop in the ACT queue
                nc.scalar.activation(dummy, G[0:1, FT - 1, 0:1], AF.Ln)
                Z = dbl.tile([P, DT, S], f32r, tag="z", bufs=1, name="Z")
                for t in range(DT):
                    ps = ps_mm.tile([P, S], f32, tag="mm", name="psf2")
                    for ft in range(FT):
                        nc.tensor.matmul(
                            ps, W2_sb[:, ft, t * P:(t + 1) * P],
                            G[:, ft, :],
                            start=(ft == 0), stop=(ft == FT - 1))
                    nc.vector.tensor_add(Z[:, t, :], ps, LN1[:, t, :])
                    nc.vector.tensor_tensor(
                        Z[:, t, :], Z[:, t, :],
                        b2_sb[:, t:t + 1].to_broadcast((P, S)), ALU.add)
                stt["Z"] = Z

            def stage_out(b, stt, nxt):
                OUT = dbl.tile([P, DT, S], f32, tag="out", name="OUT")
                filler = None
                if nxt is not None:
                    filler = lambda: stage_attn(b + 1, nxt)
                layer_norm(stt["Z"], g1_sb, nb1_sb, OUT, filler=filler)
                for t in range(DT):
                    nc.sync.dma_start(outT[b][:, t, :], OUT[:, t, :])

            # software pipeline: emit batch b+1's projections before
            # batch b's LN1 (fills the LN stats stall on the PE queue) and
            # batch b+1's attention before batch b's LN2
            sts = [dict() for _ in range(BL)]
            stage_load(0, sts[0])
            WqA_sb = load(WqA, (P, DT, 4, P))
            WkA_sb = load(WkA, (P, DT, 4, P))
            WpAq_sb = load(WpAq, (4, 4, P))
            WpAk_sb = load(WpAk, (4, 4, P))
            Wv_sb = load(Wv, (P, DT, D))
            bvb_sb = loadj(bvb, (P, D))
            Ed4_sb = load(Ed4, (P, 4, 4))
            EB4_sb = load(EB4, (4, P))
            one33_sb = load(one33, (P, 2, 33))
            onesS_sb = load(onesS, (1, S))
            g0_sb = load(g0r, (1, D))
            nb0_sb = load(nb0, (1, D))
            g1_sb = load(g1r, (1, D))
            nb1_sb = load(nb1, (1, D))
            stage_proj(0, sts[0])
            stage_attn(0, sts[0])
            W1_sb = load(W1, (P, DT, DFF))
            W2_sb = load(W2b, (P, FT, D))
            b1_sb = loadj(b1, (P, FT))
            b2_sb = loadj(b2, (P, DT))
            for b in range(BL):
                nxt = sts[b + 1] if b + 1 < BL else None
                stage_ffn(b, sts[b], nxt)
                stage_out(b, sts[b], nxt)

    nc.finalize()
    return nc


_NC = None


def kernel(Q, K, p, Wq, bq, Wk, bk, Wv, bv, Wp, bp, g0, beta0, W1, b1, W2, b2,
           g1, beta1):
    global _NC
    beta_zero = bool(np.all(np.asarray(beta0) == 0)
                     and np.all(np.asarray(beta1) == 0))
    gain_one = bool(np.all(np.asarray(g0) == 1)
                    and np.all(np.asarray(g1) == 1))
    if _NC is None:
        _NC = build_nc(beta_zero, gain_one)

    f = np.float32
    bf = ml_dtypes.bfloat16

    def feat_tiles(x):  # [B, S, D] -> [B, P, DT, S]
        x = np.asarray(x, f).transpose(0, 2, 1).reshape(-1, DT, P, S)
        return np.ascontiguousarray(x.transpose(0, 2, 1, 3))

    def pp(vec, n):  # [n*P] -> [P, n]
        return np.ascontiguousarray(np.asarray(vec, f).reshape(n, P).T)

    def wmat(w, n, m):  # [n*P, m] -> [P, n, m]
        w = np.asarray(w, f).reshape(n, P, m)
        return np.ascontiguousarray(w.transpose(1, 0, 2))

    QTf = feat_tiles(Q)
    KTf = feat_tiles(K)
    # p padded to 4 channels; row 3 = ones (carries the PE-proj bias).
    # PE projection pre-scaled by 1/4 so PhPh^T carries the 1/sqrt(DV)=1/16.
    pTf = np.zeros((B, 4, S), f)
    pTf[:, :3, :] = np.transpose(np.asarray(p, f), (0, 2, 1))
    pTf[:, 3, :] = 1.0
    # aug score weights: out tile j partitions =
    # [Qh(2j) | Ph(2j) | Qh(2j+1) | Ph(2j+1)]; pT row3==1 carries biases;
    # PE term pre-scaled by 1/4 each side so PhPh^T carries 1/sqrt(DV)=1/16
    Wq_f = np.asarray(Wq, f)
    Wk_f = np.asarray(Wk, f)
    Wp_f = np.asarray(Wp, f) * 0.25
    bq_f = np.asarray(bq, f)
    bk_f = np.asarray(bk, f)
    bp_f = np.asarray(bp, f) * 0.25

    def aug_w(W):  # [D, D] -> [P, DT, 4, P] lhsT tiles
        out = np.zeros((P, DT, 4, P), f)
        Wt = W.reshape(DT, P, D)  # [kt, row, out_feature]
        for j in range(4):
            for hh in range(2):
                h = 2 * j + hh
                out[:, :, j, 64 * hh:64 * hh + 32] = \
                    Wt[:, :, 32 * h:32 * h + 32].transpose(1, 0, 2)
        return out

    def aug_p(bias):  # [4, 4, P]: rows 0-2 Wp at P slots, row 3 biases
        out = np.zeros((4, 4, P), f)
        for j in range(4):
            for hh in range(2):
                h = 2 * j + hh
                out[:3, j, 64 * hh + 32:64 * hh + 64] = \
                    Wp_f[:, 32 * h:32 * h + 32]
                out[3, j, 64 * hh:64 * hh + 32] = bias[32 * h:32 * h + 32]
                out[3, j, 64 * hh + 32:64 * hh + 64] = \
                    bp_f[32 * h:32 * h + 32]
        return out

    # EB4: r4 row h4 -> out partitions 32*h4..32*h4+31
    EB4m = np.zeros((4, P), f)
    for h4 in range(4):
        EB4m[h4, 32 * h4:32 * h4 + 32] = 1.0
    # Ed4[:, h4, :]: all-ones col h4 (masked partition-sum lhsT)
    Ed4m = np.zeros((P, 4, 4), f)
    for h4 in range(4):
        Ed4m[:, h4, h4] = 1.0
    # LN partition-sum weights (1/D folded in): [:,0,:] puts sum(x)/D at
    # out partition 0, [:,1,:] puts sum(x^2)/D at out partition 32
    one33m = np.zeros((P, 2, 33), f)
    one33m[:, 0, 0] = 1.0 / D
    one33m[:, 1, 32] = 1.0 / D

    shared = {
        "Wq": wmat(Wq, DT, D).astype(bf),
        "Wv": wmat(Wv, DT, D).astype(bf),
        "WqA": aug_w(Wq_f).astype(bf), "WkA": aug_w(Wk_f).astype(bf),
        "WpAq": aug_p(bq_f).astype(bf), "WpAk": aug_p(bk_f).astype(bf),
        "W1": wmat(W1, DT, DFF).astype(bf),
        "W2b": wmat(W2, FT, D).astype(bf),
        "bq": pp(bq, DT),
        "bvb": np.ascontiguousarray(np.broadcast_to(np.asarray(bv, f), (P, D))),
        "b1": pp(b1, FT), "b2": pp(b2, DT),
        "g0r": np.asarray(g0, f).reshape(1, D),
        "nb0": -np.asarray(beta0, f).reshape(1, D),
        "g1r": np.asarray(g1, f).reshape(1, D),
        "nb1": -np.asarray(beta1, f).reshape(1, D),
        "one33": one33m,
        "Ed4": Ed4m.astype(bf), "EB4": EB4m,
        "onesS": np.ones((1, S), f),
    }
    in_maps = []
    for c in range(NCORES):
        m = dict(shared)
        m["QT"] = np.ascontiguousarray(QTf[c * BL:(c + 1) * BL]).astype(bf)
        m["KT"] = np.ascontiguousarray(KTf[c * BL:(c + 1) * BL]).astype(bf)
        m["pT"] = np.ascontiguousarray(pTf[c * BL:(c + 1) * BL]).astype(bf)
        in_maps.append(m)

    import os
    trace = bool(os.environ.get("BASS_TRACE"))
    res = run_bass_kernel_spmd(_NC, in_maps, core_ids=list(range(NCORES)),
                               trace=trace)
    kernel._LAST = res
    outs = [res.results[c]["outT"] for c in range(NCORES)]
    full = np.concatenate(outs, axis=0)  # [B, P, DT, S]
    full = full.transpose(0, 2, 1, 3).reshape(B, D, S)  # [B, D, S]
    return np.ascontiguousarray(full.transpose(0, 2, 1))
